# revision 1
# baseline (speedup 1.0000x reference)
"""AttentionSubsample Trainium2 kernel — data-parallel over batch on 8 cores.

v4. On top of v3 (strided-identity transposes, 98/98 strips, quad-level B2):
  - 5-stage interleaved issue: A(k) | scores(k-1) | attn@v(k-2) |
    transposes(k-1) | y/proj/out(k-2).  attn@v MMs fill the PE gap where
    transposes would otherwise wait on exp (ACT), raising PE duty so the
    HAM clock gate stays at 2.4GHz (v3 trace: K=4/8 half the kernel).
  - scores into 2-bank PSUM tiles (4 heads), one exp per tile (FD=784).
  - hardswish chain (y+tvb, relu(y+3) clamp, min*y) on GpSimd — SBUF-only
    ops on the otherwise idle engine.
  - const DMA order: A-phase consts first; stk blobs via gpsimd SWDGE.
"""

import numpy as np
import ml_dtypes

import concourse.bass as bass
import concourse.tile as tile
from concourse import bacc, mybir
from concourse.bass_utils import run_bass_kernel_spmd

BF16 = mybir.dt.bfloat16
F32 = mybir.dt.float32
F8 = mybir.dt.float8e4
WS = 32.0          # fp8 weight scale; folded out via exp scale & v evac

B, N, NQ, C = 512, 196, 49, 256
H = 8
NCORES = 8
BPC = B // NCORES
EPS = 1e-5
SCALE = 16 ** -0.5
AF = mybir.ActivationFunctionType
ALU = mybir.AluOpType

bf16 = ml_dtypes.bfloat16
STAGE_MARKS = []


def build_core(nbatch=BPC):
    assert nbatch % 4 == 0
    nc = bacc.Bacc("TRN2", target_bir_lowering=False, debug=False)

    xt_d = nc.dram_tensor("xt", [nbatch, 2, 128, N], F8, kind="ExternalInput")
    wkt_d = nc.dram_tensor("wkt", [2, 128, 128], F8, kind="ExternalInput")
    wqt_d = nc.dram_tensor("wqt", [2, 128, 128], F8, kind="ExternalInput")
    wvt_d = nc.dram_tensor("wvt", [2, 128, 256], F8, kind="ExternalInput")
    wpt_d = nc.dram_tensor("wpt", [2, 128, 512], BF16, kind="ExternalInput")
    tkq_d = nc.dram_tensor("tkq", [128, 245], BF16, kind="ExternalInput")
    stk_d = nc.dram_tensor("stk", [49, 8, 4, 245], BF16, kind="ExternalInput")
    id128_d = nc.dram_tensor("id128", [128, 128], BF16, kind="ExternalInput")
    tvb_d = nc.dram_tensor("tvb", [128, 256], F32, kind="ExternalInput")
    out_d = nc.dram_tensor("out", [nbatch, 49, 512], BF16, kind="ExternalOutput")
    outv = out_d.rearrange("(x pr b2) q o -> b2 x q pr o", pr=2, b2=2)

    with tile.TileContext(nc) as tc:
        with (
            tc.tile_pool(name="consts", bufs=1) as consts,
            tc.tile_pool(name="io", bufs=4) as io,
            tc.tile_pool(name="work", bufs=8) as work,
            tc.tile_pool(name="attnp", bufs=4) as attnp,
            tc.tile_pool(name="stackp", bufs=1) as stackp,
            tc.tile_pool(name="ps_kv", bufs=2, space="PSUM") as ps_kv,
            tc.tile_pool(name="ps_s", bufs=2, space="PSUM") as ps_s,
            tc.tile_pool(name="ps_op", bufs=1, space="PSUM") as ps_op,
            # 6-stage skew: A(it) | scores(it-1) | hsT+proj+out(it-3) |
            # transp(it-1) | attn@v + y-chain(it-2)
        ):
            # A-phase consts first so quad 0 is not gated on B-phase blobs
            wkt_sb = consts.tile([128, 2, 128], F8)
            wqt_sb = consts.tile([128, 2, 128], F8)
            wvt_sb = consts.tile([128, 2, 256], F8)
            wpt_sb = consts.tile([128, 2, 512], BF16)
            tkq_sb = consts.tile([128, 245], BF16)
            tvb_sb = consts.tile([128, 256], F32)
            id128_sb = consts.tile([128, 128], BF16)
            for c in range(2):
                nc.scalar.dma_start(out=wkt_sb[:, c, :], in_=wkt_d[c])
                nc.scalar.dma_start(out=wqt_sb[:, c, :], in_=wqt_d[c])
            nc.scalar.dma_start(out=tkq_sb, in_=tkq_d[:])
            for c in range(2):
                nc.scalar.dma_start(out=wvt_sb[:, c, :], in_=wvt_d[c])
            nc.scalar.dma_start(out=tvb_sb, in_=tvb_d[:])
            nc.scalar.dma_start(out=id128_sb, in_=id128_d[:])
            for c in range(2):
                nc.scalar.dma_start(out=wpt_sb[:, c, :], in_=wpt_d[c])
            qka_tiles = []
            for i_ in range(2):
                t = stackp.tile([65, 8, 4, 245], BF16, tag=f"qka{i_}")
                nc.gpsimd.dma_start(out=t[16:65, :, :, :], in_=stk_d[:])
                qka_tiles.append(t)
            v_tiles = []
            for i_ in range(4):
                quad = []
                for j_ in range(4):
                    v0 = stackp.tile([98, 8, 33], BF16, tag=f"v0_{i_}{j_}")
                    v1 = stackp.tile([98, 8, 33], BF16, tag=f"v1_{i_}{j_}")
                    nc.vector.memset(v0[:, :, 32:33], 1.0)
                    nc.vector.memset(v1[:, :, 32:33], 1.0)
                    quad.append((v0, v1))
                v_tiles.append(quad)

            # identity view with only the useful 98 query columns
            idq = id128_sb[0:113, :].rearrange(
                "p (two q) -> p two q", two=2)[:, :, 0:49]
            three_sb = consts.tile([128, 1], F32)
            nc.vector.memset(three_sb, 3.0)

            DR = mybir.MatmulPerfMode.DoubleRow

            def xt_load(qd):
                # prefetch at iteration top; consumed by phase_a at the
                # iteration tail.  free dim padded to 208 so the DoubleRow
                # Ko-pair stride (208 fp8 bytes) is 16-byte aligned
                xt_sb = io.tile([128, 8, 208], F8)
                nc.gpsimd.dma_start(
                    out=xt_sb[:, :, 0:196],
                    in_=xt_d[4 * qd:4 * qd + 4].rearrange(
                        "b c q n -> q (b c) n"),
                )
                return xt_sb

            def phase_a(qd, xt_sb):
                def xs_view(qb):
                    v = xt_sb[:, 2 * qb:2 * qb + 2, 0:196].rearrange(
                        "q c (a s c2 t) -> q c a s c2 t", a=7, s=2, c2=7, t=2
                    )
                    return v[:, :, :, 0, :, 0]

                qkT_sb = work.tile([128, 4, 245], BF16)
                v0_sbs, v1_sbs = [], []
                for qb in range(4):
                    # kq in bank 0, v in bank 1: the v matmuls don't WAR-wait
                    # on the kq evacuation
                    kv_ps = ps_kv.tile([128, 2, 512], F32)
                    nc.tensor.matmul(
                        kv_ps[:, 0, 0:196], lhsT=wkt_sb,
                        rhs=xt_sb[:, 2 * qb:2 * qb + 2, 0:196],
                        start=True, stop=True, perf_mode=DR,
                    )
                    nc.tensor.matmul(
                        kv_ps[:, 0, 196:245], lhsT=wqt_sb,
                        rhs=xs_view(qb),
                        start=True, stop=True, perf_mode=DR,
                    )
                    nc.vector.tensor_add(
                        qkT_sb[:, qb, :], kv_ps[:, 0, 0:245], tkq_sb)

                    nc.tensor.matmul(
                        kv_ps[0:98, 1, 0:256],
                        lhsT=xt_sb[:, 2 * qb:2 * qb + 2, 0:98],
                        rhs=wvt_sb,
                        start=True, stop=True, perf_mode=DR,
                    )
                    nc.tensor.matmul(
                        kv_ps[0:98, 1, 256:512],
                        lhsT=xt_sb[:, 2 * qb:2 * qb + 2, 98:196],
                        rhs=wvt_sb,
                        start=True, stop=True, perf_mode=DR,
                    )
                    v0_sb, v1_sb = v_tiles[qd % 4][qb]
                    nc.vector.tensor_scalar_mul(
                        v0_sb[:, :, 0:32],
                        kv_ps[0:98, 1, 0:256].rearrange(
                            "q (h d) -> q h d", h=8),
                        1.0 / WS)
                    nc.scalar.activation(
                        v1_sb[:, :, 0:32],
                        kv_ps[0:98, 1, 256:512].rearrange(
                            "q (h d) -> q h d", h=8),
                        AF.Copy, scale=1.0 / WS)
                    v0_sbs.append(v0_sb)
                    v1_sbs.append(v1_sb)

                qka_sb = qka_tiles[qd % 2]
                for h in range(H):
                    nc.sync.dma_start(
                        out=qka_sb[0:16, h, :, :],
                        in_=qkT_sb[16 * h:16 * h + 16, :, :])
                return qka_sb, v0_sbs, v1_sbs

            def b1_scores(qd, qka_sb, v0_sbs, v1_sbs):
                # scores into 2-bank tiles (4 heads each), one exp per tile
                attn_tiles = []
                for _pr in range(2):
                    attn_t = attnp.tile([128, 8, 196], BF16, tag="attn")
                    attn_tiles.append(attn_t)
                for pr in range(2):
                    for j in range(4):
                        s_ps = ps_s.tile([113, 512], F32, tag="s")
                        for jj in range(2):
                            h = 2 * j + jj
                            for b2 in range(2):
                                nc.tensor.matmul(
                                    s_ps[64 * b2:64 * b2 + 49,
                                         196 * jj:196 * jj + 196],
                                    lhsT=qka_sb[:, h, 2 * pr + b2, 196:245],
                                    rhs=qka_sb[:, h, 2 * pr + b2, 0:196],
                                    start=True, stop=True,
                                    tile_position=(0, 64 * b2),
                                )
                        nc.scalar.activation(
                            out=attn_tiles[pr][0:113, 2 * j:2 * j + 2, 0:196],
                            in_=s_ps[:, 0:392].rearrange(
                                "q (jj n) -> q jj n", jj=2),
                            func=AF.Exp, scale=1.0 / (WS * WS),
                        )
                return attn_tiles

            def stage_tav(jst, kst):
                """Transposes of quad j zipped with attn@v of quad k.

                Transpose-mode doesn't register as PE activity for the HAM
                clock gate; a 16-transpose burst re-throttles the clock to
                1.2GHz.  Interleaving real matmuls keeps it at 2.4GHz.
                """
                aT_tiles = None
                if jst is not None:
                    attn_tiles = jst
                    aT_tiles = []
                    for _pr in range(2):
                        aT_t = attnp.tile([98, 8, 2, 2, 49], BF16, tag="aT")
                        aT_tiles.append(aT_t)
                op_ps = None
                if kst is not None:
                    k_aT_tiles, (_, v0_sbs, v1_sbs) = kst
                    op_ps = ps_op.tile([128, 2, 512], F32)
                for pr in range(2):
                    if jst is not None:
                        attn_sb, aT_sb = attn_tiles[pr], aT_tiles[pr]
                        taT0 = ps_s.tile([98, 8, 2, 49], BF16, tag="s")
                        taT1 = ps_s.tile([98, 8, 2, 49], BF16, tag="s")
                    for h in range(H):
                        if jst is not None:
                            nc.tensor.transpose(
                                taT0[:, h, :, :], attn_sb[0:113, h, 0:98],
                                idq)
                            nc.tensor.transpose(
                                taT1[:, h, :, :], attn_sb[0:113, h, 98:196],
                                idq)
                        if kst is not None:
                            aTk = k_aT_tiles[pr]
                            for s in range(2):
                                vs = (v0_sbs, v1_sbs)[s]
                                for b2 in range(2):
                                    nc.tensor.matmul(
                                        op_ps[64 * b2:64 * b2 + 49, pr,
                                              33 * h:33 * h + 33],
                                        lhsT=aTk[:, h, s, b2, :],
                                        rhs=vs[2 * pr + b2][:, h, :],
                                        start=(s == 0), stop=(s == 1),
                                        tile_position=(0, 64 * b2),
                                    )
                    if jst is not None:
                        nc.vector.tensor_copy(aT_sb[:, :, 0, :, :], taT0)
                        nc.vector.tensor_copy(aT_sb[:, :, 1, :, :], taT1)
                if kst is None:
                    return aT_tiles, None

                # y-chain issues with attn@v; consumers run next iteration
                o_view = op_ps[:, :, 0:264].rearrange(
                    "q pr (h d) -> q pr h d", h=8)
                zr_sb = work.tile([113, 2, 8], F32, tag="tmp")
                nc.vector.reciprocal(zr_sb, o_view[0:113, :, :, 32])
                zr_b = bass.AP(tensor=zr_sb.tensor, offset=zr_sb.offset,
                               ap=[zr_sb.ap[0], zr_sb.ap[1], zr_sb.ap[2],
                                   [0, 32]])
                tvb_h = tvb_sb.rearrange("q (h d) -> q h d", h=8)
                tvb_b = bass.AP(tensor=tvb_h.tensor, offset=tvb_h.offset,
                                ap=[[tvb_h.ap[0][0], 113], [0, 2],
                                    tvb_h.ap[1], tvb_h.ap[2]])
                y_sb = work.tile([113, 2, 8, 32], F32, tag="y")
                nc.vector.tensor_mul(y_sb, o_view[0:113, :, :, 0:32], zr_b)
                nc.gpsimd.tensor_add(y_sb, y_sb, tvb_b)
                y_flat = y_sb.rearrange("q pr h d -> q pr (h d)")
                r_sb = work.tile([113, 2, 256], F32, tag="tmp")
                nc.scalar.activation(
                    r_sb, y_flat, AF.Relu, bias=three_sb[0:113, :], scale=1.0)
                hs_sb = work.tile([128, 2, 256], BF16, tag="hs")
                nc.vector.scalar_tensor_tensor(
                    out=hs_sb[0:113, :, :], in0=r_sb, scalar=6.0,
                    in1=y_flat, op0=ALU.min, op1=ALU.mult,
                )
                return aT_tiles, (op_ps, hs_sb)

            def b2_fin(qd, op_ps, hs_sb):
                for pr in range(2):
                    thsT = ps_s.tile([128, 2, 2, 49], BF16, tag="s")
                    for cc in range(2):
                        nc.tensor.transpose(
                            thsT[:, cc, :, :],
                            hs_sb[0:113, pr, 128 * cc:128 * cc + 128], idq)
                    hsT_sb = work.tile([128, 2, 2, 49], BF16, tag="hsT")
                    nc.vector.tensor_copy(hsT_sb, thsT)
                    for cc in range(2):
                        for b2 in range(2):
                            nc.tensor.matmul(
                                op_ps[64 * b2:64 * b2 + 49, pr, 0:512],
                                lhsT=hsT_sb[:, cc, b2, :],
                                rhs=wpt_sb[:, cc, :],
                                start=(cc == 0), stop=(cc == 1),
                                tile_position=(0, 64 * b2),
                            )
                out_sb = io.tile([113, 2, 512], BF16)
                nc.scalar.activation(out_sb, op_ps[0:113, :, :], AF.Copy)
                for b2 in range(2):
                    nc.gpsimd.dma_start(
                        out=outv[b2, qd],
                        in_=out_sb[64 * b2:64 * b2 + 49, :, :])

            def mark(stage, qd):
                STAGE_MARKS.append(
                    (stage, qd,
                     int(nc.get_next_instruction_name().split("-")[1])))

            # block order inside an iteration follows per-engine readiness:
            # stages whose inputs are a full iteration old go first, the
            # A-stage (whose evacs only become ready here) goes last
            nq = nbatch // 4
            x_st, a_st, s_st, t_st, o_st = {}, {}, {}, {}, {}
            for it in range(nq + 3):
                if it + 1 < nq:
                    mark("xt", it + 1)
                    x_st[it + 1] = xt_load(it + 1)
                if it < nq:
                    mark("A", it)
                    if it == 0:
                        x_st[0] = xt_load(0)
                    a_st[it] = phase_a(it, x_st.pop(it))
                j, k, m = it - 1, it - 2, it - 3
                if 0 <= j < nq:
                    mark("scores", j)
                    s_st[j] = b1_scores(j, *a_st[j])
                if 0 <= m < nq:
                    mark("fin", m)
                    b2_fin(m, *o_st.pop(m))
                jst = s_st.pop(j) if 0 <= j < nq else None
                kst = (t_st.pop(k), a_st.pop(k)) if 0 <= k < nq else None
                if jst is not None or kst is not None:
                    mark("tav", it)
                    aT, ost = stage_tav(jst, kst)
                    if aT is not None:
                        t_st[j] = aT
                    if ost is not None:
                        o_st[k] = ost

    STAGE_MARKS.append(("end", -1,
                        int(nc.get_next_instruction_name().split("-")[1])))
    nc.compile()
    return nc


def _build_bias_idxs():
    import itertools
    points = list(itertools.product(range(14), range(14)))
    points_ = list(itertools.product(range(7), range(7)))
    offsets, idxs = {}, []
    for p1 in points_:
        for p2 in points:
            off = (abs(p1[0] * 2 - p2[0]), abs(p1[1] * 2 - p2[1]))
            if off not in offsets:
                offsets[off] = len(offsets)
            idxs.append(offsets[off])
    return np.array(idxs, dtype=np.int32).reshape(NQ, N)


def make_inputs(x, w_kv, kv_g, kv_b, kv_m, kv_v, w_q, q_g, q_b, q_m, q_v,
                w_p, p_g, p_b, p_m, p_v, ab_table, bias_idxs, nbatch=BPC,
                ncores=NCORES):
    """Host-side preprocessing -> list of per-core input dicts."""
    f = np.float32
    x = np.asarray(x, f)
    s_kv = np.asarray(kv_g, f) / np.sqrt(np.asarray(kv_v, f) + EPS)
    wkv = np.asarray(w_kv, f) * s_kv[:, None]
    tkv = np.asarray(kv_b, f) - np.asarray(kv_m, f) * s_kv
    wkv_h = wkv.reshape(H, 48, C)
    tkv_h = tkv.reshape(H, 48)
    w_k = wkv_h[:, :16, :].reshape(128, C)
    t_k = tkv_h[:, :16].reshape(128)
    w_v = wkv_h[:, 16:, :].reshape(256, C)
    t_v = tkv_h[:, 16:].reshape(256)

    s_q = np.asarray(q_g, f) / np.sqrt(np.asarray(q_v, f) + EPS)
    wq = np.asarray(w_q, f) * (s_q * SCALE)[:, None]
    t_q = (np.asarray(q_b, f) - np.asarray(q_m, f) * s_q) * SCALE

    s_p = np.asarray(p_g, f) / np.sqrt(np.asarray(p_v, f) + EPS)
    wp = np.asarray(w_p, f) * s_p[:, None] / 6.0
    t_p = np.asarray(p_b, f) - np.asarray(p_m, f) * s_p

    idxs = _build_bias_idxs()
    WS = 32.0
    ab = np.asarray(ab_table, f)[:, idxs]                       # [8,49,196]
    ab_s = ab.transpose(1, 0, 2) * (WS * WS)                    # [49,8,196]
    qa_c = np.broadcast_to(np.eye(NQ, dtype=f)[:, None, :], (NQ, H, NQ))
    stk1 = np.concatenate([ab_s, qa_c], axis=2)                 # [49,8,245]
    stk = np.ascontiguousarray(
        np.broadcast_to(stk1[:, :, None, :], (NQ, H, 4, 245)))

    tkq = np.concatenate(
        [np.broadcast_to(t_k[:, None], (128, 196)),
         np.broadcast_to(t_q[:, None], (128, 49))], axis=1) * WS

    f8 = ml_dtypes.float8_e4m3fn
    base = dict(
        wkt=np.ascontiguousarray(w_k.T.reshape(2, 128, 128) * WS).astype(f8),
        wqt=np.ascontiguousarray(wq.T.reshape(2, 128, 128) * WS).astype(f8),
        wvt=np.ascontiguousarray(w_v.T.reshape(2, 128, 256) * WS).astype(f8),
        wpt=np.ascontiguousarray(wp.T.reshape(2, 128, 512)).astype(bf16),
        tkq=np.ascontiguousarray(tkq).astype(bf16),
        stk=stk.astype(bf16),
        id128=np.eye(128, dtype=f).astype(bf16),
        tvb=np.ascontiguousarray(np.broadcast_to(t_v, (128, 256))),
    )

    xt = x.transpose(0, 2, 1).astype(f8).reshape(B, 2, 128, N)
    in_maps = []
    for cid in range(ncores):
        m = dict(base)
        m["xt"] = np.ascontiguousarray(xt[cid * nbatch:(cid + 1) * nbatch])
        in_maps.append(m)
    return in_maps, t_p


_NC_CACHE = {}
LAST_RESULT = None


def kernel(**inputs):
    if "nc" not in _NC_CACHE:
        _NC_CACHE["nc"] = build_core(BPC)
    nc = _NC_CACHE["nc"]
    in_maps, t_p = make_inputs(**inputs)
    res = run_bass_kernel_spmd(nc, in_maps, core_ids=list(range(NCORES)))
    global LAST_RESULT
    LAST_RESULT = res
    out = np.concatenate([r["out"] for r in res.results], axis=0)
    return out.astype(np.float32) + t_p

